# revision 27
# baseline (speedup 1.0000x reference)
"""GPTQ int4 dequant + GEMM  (M=32, K=8192, N=8192, group=64) on 8 TRN2 cores.

Strategy
--------
Tensor-parallel over out_features N (1024 per core), x replicated.

The kernel is HBM-bound, so the win is shipping fewer weight bytes.  The
smallest PE-consumable dtype with enough mantissa is float8e3 (e3m4):

  host:   w = (q - zeros[g]) * scales[g];  per-channel fold S[n] = max|w|/15.5
          w8 = e3m4(w / S[n])  -> 1 B/weight, rel err ~1.4% (gate is 2e-2)
          x^T packed bf16 (mixed-dtype matmul is legal on TRN2)
  device: acc[m, n] = sum_k x^T[k, m] * w8^T[k, n]   (PSUM f32)
          4-way PE column tiling (M=32 uses 32 of 128 array cols): col group
          j = tile % 4 accumulates its tiles into PSUM rows 32j..32j+31.
          Eviction: DVE (bank0) || ACT (bank1, table preloaded), bf16 out.
  host:   out = (P0+P1+P2+P3) * S + bias; concat the 8 N-shards

HW quirks this kernel works around (all probe/trace-verified):
  * x rides the scalar-HWDGE queue so its completion receipt overlaps
    chunk 0's data on the sync queue (PE starts ~2us earlier).
  * HAM clock gate: dummy matmuls between chunk waits keep the PE at 2.4 GHz.
  * The final DMA-receipt wait stays: dropping it intermittently ships a
    stale output buffer (host readback can race the in-flight write).
  * Partition-sliced DMAs ([0:113) etc.) fall off the HWDGE fast path; a
    P-partition DMA fans out over the largest divisor of P <= 16 engines
    (tried mixed K=120 tiles to dodge the ~10% slower engine 15, but the
    15-engine fan-out throttles aggregate rate — uniform K=128 wins).
"""

import numpy as np
import ml_dtypes

M, K, N = 32, 8192, 8192
GROUP_SIZE = 64
N_CORES = 8
NC = N // N_CORES            # 1024 out-features per core
E3M4_MAX = 15.5

# uniform k-tiling: 64 tiles of K=128
NT128, NT120 = 64, 0
NTILES = NT128 + NT120
ROWS128 = NT128 * 128
# chunk schedule: (kind, first_tile, n_tiles) — 16 x 512 KiB weight DMAs
CHUNKS = [(128, 4 * i, 4) for i in range(16)]

_cached = {}


def _tile_k(t):
    return 128 if t < NT128 else 120


def _build_program():
    from contextlib import ExitStack

    import concourse.bass as bass
    import concourse.mybir as mybir

    bf16 = mybir.dt.bfloat16
    f8e3 = mybir.dt.float8e3
    f32 = mybir.dt.float32

    nc = bass.Bass()
    # w128[p][t*NC + n] = w8T[128t + p, n] for t in [0, 49)
    w128_ext = nc.declare_dram_parameter("w128", [128, NT128 * NC], f8e3,
                                         isOutput=False)
    # w120[p][i*NC + n] = w8T[6272 + 120i + p, n] for i in [0, 16)
    w120_ext = (nc.declare_dram_parameter("w120", [120, NT120 * NC], f8e3,
                                          isOutput=False)
                if NT120 else None)
    # xTp[p, t*M + m] = x[m, krow(t, p)]  (bf16)
    x_ext = nc.declare_dram_parameter("xTp", [128, NTILES * M], bf16,
                                      isOutput=False)
    o_ext = nc.declare_dram_parameter("out", [128, NC], bf16, isOutput=True)

    with ExitStack() as ctx:
        wbuf = ctx.enter_context(nc.sbuf_tensor([128, NTILES * NC], f8e3))
        xbuf = ctx.enter_context(nc.sbuf_tensor([128, NTILES * M], bf16))
        obuf = ctx.enter_context(nc.sbuf_tensor([128, NC], bf16))
        scratch = ctx.enter_context(nc.sbuf_tensor([1, 8], f32))
        ps0 = ctx.enter_context(nc.psum_tensor([128, 512], f32))
        ps1 = ctx.enter_context(nc.psum_tensor([128, 512], f32))
        ps2 = ctx.enter_context(nc.psum_tensor([128, 512], f32))
        xsem = ctx.enter_context(nc.semaphore())
        wsems = [ctx.enter_context(nc.semaphore(name=f"wsem{i}"))
                 for i in range(len(CHUNKS))]
        wtail = ctx.enter_context(nc.semaphore())
        pesem = ctx.enter_context(nc.semaphore())
        vsem = ctx.enter_context(nc.semaphore())
        osem = ctx.enter_context(nc.semaphore())
        block = ctx.enter_context(nc.Block(no_gpsimd_drain=True))

        @block.sync
        def _(sync):
            for c, (kind, t0, nt) in enumerate(CHUNKS):
                if kind == 128:
                    src = w128_ext[:, t0 * NC:(t0 + nt) * NC]
                    dst = wbuf[:, t0 * NC:(t0 + nt) * NC]
                else:
                    i0 = t0 - NT128
                    src = w120_ext[:, i0 * NC:(i0 + nt) * NC]
                    dst = wbuf[0:120, t0 * NC:(t0 + nt) * NC]
                if c == len(CHUNKS) - 1:
                    # finer tail: last chunk lands as two 2-tile pieces
                    half = (nt // 2) * NC
                    sync.dma_start(out=dst[:, 0:half],
                                   in_=src[:, 0:half]).then_inc(wsems[c], 16)
                    sync.dma_start(out=dst[:, half:],
                                   in_=src[:, half:]).then_inc(wtail, 16)
                else:
                    sync.dma_start(out=dst, in_=src).then_inc(wsems[c], 16)
            # bank0 out-DMA as soon as DVE evicted it; bank1 goes out on the
            # scalar queue; wait for both write receipts before retiring
            sync.wait_ge(vsem, 1)
            sync.dma_start(out=o_ext[:, 0:512],
                           in_=obuf[:, 0:512]).then_inc(osem, 16)
            sync.wait_ge(osem, 32)

        @block.scalar
        def _(scalar):
            # x on the ACT HWDGE queue: its completion receipt overlaps
            # chunk0's data on the sync queue
            scalar.dma_start(out=xbuf[:], in_=x_ext[:]).then_inc(xsem, 16)
            # dummy op: pay the ACT table load during the DMA stream
            scalar.copy(scratch[:], scratch[:])
            scalar.wait_ge(pesem, 2)
            scalar.copy(obuf[:, 512:1024], ps1[:])
            scalar.dma_start(out=o_ext[:, 512:1024],
                             in_=obuf[:, 512:1024]).then_inc(osem, 16)

        @block.tensor
        def _(tensor):
            psd = ps2[0:32, :]

            def dummy_mms(n):
                # HAM warm-keepers: fill PE wait-gaps with throwaway matmuls
                # so the activity monitor holds the 2.4 GHz clock.
                for _ in range(n):
                    tensor.matmul(psd, xbuf[:, 0:M], wbuf[:, 0:512],
                                  start=True, stop=True,
                                  tile_position=(0, 0))

            tensor.wait_ge(xsem, 16)
            dummy_mms(8)
            nch = len(CHUNKS)
            for c, (kind, t0, nt) in enumerate(CHUNKS):
                tensor.wait_ge(wsems[c], 16)
                for ti in range(nt):
                    if c == nch - 1 and ti == nt // 2:
                        tensor.wait_ge(wtail, 16)
                    t = t0 + ti
                    kk = _tile_k(t)
                    j = t % 4
                    lhsT = xbuf[0:kk, t * M:(t + 1) * M]
                    w_off = t * NC
                    start = t < 4
                    stop = t >= NTILES - 4
                    mm0 = tensor.matmul(ps0[32 * j:32 * j + 32, :], lhsT,
                                        wbuf[0:kk, w_off:w_off + 512],
                                        start=start, stop=stop,
                                        tile_position=(0, 32 * j))
                    mm1 = tensor.matmul(ps1[32 * j:32 * j + 32, :], lhsT,
                                        wbuf[0:kk, w_off + 512:w_off + 1024],
                                        start=start, stop=stop,
                                        tile_position=(0, 32 * j))
                    if t == NTILES - 1:
                        mm0.then_inc(pesem, 1)
                        mm1.then_inc(pesem, 1)
                if c < nch - 2:
                    dummy_mms(2)

        @block.vector
        def _(vector):
            vector.wait_ge(pesem, 1)
            vector.tensor_copy(obuf[:, 0:512], ps0[:]).then_inc(vsem, 1)

    return nc


def _host_prep(x, packed_weight, scales, zeros):
    """Dequantize, fold per-channel scale, quantize to e3m4, pack layouts."""
    bf16 = ml_dtypes.bfloat16
    e3m4 = ml_dtypes.float8_e3m4
    k = np.arange(K)
    shift = ((k % 2) * 4).astype(np.int32)
    q = ((packed_weight[:, k // 2] >> shift[None, :]) & 15).astype(np.float32)
    g = k // GROUP_SIZE
    w = (q - zeros[:, g]) * scales[:, g]            # [N, K] f32
    S = np.abs(w).max(axis=1) / E3M4_MAX            # [N]
    w8 = (w / S[:, None]).astype(e3m4)              # [N, K] e3m4

    # x^T packed by tile: [128, NTILES*M]
    xTp = np.zeros((128, NTILES * M), np.float32)
    xT = x.T                                        # [K, M]
    for t in range(NTILES):
        kk = _tile_k(t)
        r0 = 128 * t if t < NT128 else ROWS128 + 120 * (t - NT128)
        xTp[0:kk, t * M:(t + 1) * M] = xT[r0:r0 + kk]
    xTp = xTp.astype(bf16)

    in_maps = []
    for c in range(N_CORES):
        wc = w8[c * NC:(c + 1) * NC].T              # [K, NC] e3m4 view
        w128 = np.ascontiguousarray(
            wc[0:ROWS128].reshape(NT128, 128, NC).transpose(1, 0, 2)
              .reshape(128, NT128 * NC))
        m = {"w128": w128, "xTp": xTp}
        if NT120:
            m["w120"] = np.ascontiguousarray(
                wc[ROWS128:].reshape(NT120, 120, NC).transpose(1, 0, 2)
                  .reshape(120, NT120 * NC))
        in_maps.append(m)
    return in_maps, S


def kernel(x, packed_weight, scales, zeros, bias_param, _trace=False):
    from concourse.bass_utils import run_bass_kernel_spmd

    if "nc" not in _cached:
        _cached["nc"] = _build_program()
    nc = _cached["nc"]

    in_maps, S = _host_prep(x, packed_weight, scales, zeros)
    res = run_bass_kernel_spmd(nc, in_maps, core_ids=list(range(N_CORES)),
                               trace=_trace)
    parts = []
    for c in range(N_CORES):
        P = res.results[c]["out"].astype(np.float32)    # [128, NC]
        acc = P[0:32] + P[32:64] + P[64:96] + P[96:128]
        parts.append(acc * S[None, c * NC:(c + 1) * NC])
    out = np.concatenate(parts, axis=1) + bias_param[None, :].astype(np.float32)
    out = out.astype(np.float32, copy=False)
    if _trace:
        return out, res
    return out


# revision 28
# speedup vs baseline: 1.0943x; 1.0943x over previous
"""GPTQ int4 dequant + GEMM  (M=32, K=8192, N=8192, group=64) on 8 TRN2 cores.

Strategy
--------
Tensor-parallel over out_features N (1024 per core), x replicated.

The kernel is HBM-bound, so the win is shipping fewer weight bytes.  The
smallest PE-consumable dtype with enough mantissa is float8e3 (e3m4):

  host:   w = (q - zeros[g]) * scales[g];  per-channel fold S[n] = max|w|/15.5
          w8 = e3m4(w / S[n])  -> 1 B/weight, rel err ~1.4% (gate is 2e-2)
          x^T packed bf16 (mixed-dtype matmul is legal on TRN2)
  device: acc[m, n] = sum_k x^T[k, m] * w8^T[k, n]   (PSUM f32)
          4-way PE column tiling (M=32 uses 32 of 128 array cols): col group
          j = tile % 4 accumulates its tiles into PSUM rows 32j..32j+31.
          Eviction: DVE (bank0) || ACT (bank1, table preloaded), bf16 out.
  host:   out = (P0+P1+P2+P3) * S + bias; concat the 8 N-shards

HW quirks this kernel works around (all probe/trace-verified):
  * x rides the scalar-HWDGE queue so its completion receipt overlaps
    chunk 0's data on the sync queue (PE starts ~2us earlier).
  * HAM clock gate: dummy matmuls between chunk waits keep the PE at 2.4 GHz.
  * The final DMA-receipt wait stays: dropping it intermittently ships a
    stale output buffer (host readback can race the in-flight write).
  * Partition-sliced DMAs ([0:113) etc.) fall off the HWDGE fast path; a
    P-partition DMA fans out over the largest divisor of P <= 16 engines
    (tried mixed K=120 tiles to dodge the ~10% slower engine 15, but the
    15-engine fan-out throttles aggregate rate — uniform K=128 wins).
"""

import numpy as np
import ml_dtypes

M, K, N = 32, 8192, 8192
GROUP_SIZE = 64
N_CORES = 8
NC = N // N_CORES            # 1024 out-features per core
E3M4_MAX = 15.5

# uniform k-tiling: 64 tiles of K=128
NT128, NT120 = 64, 0
NTILES = NT128 + NT120
ROWS128 = NT128 * 128
# chunk schedule: (kind, first_tile, n_tiles) — 8 x 1 MiB weight DMAs
CHUNKS = [(128, 8 * i, 8) for i in range(8)]

_cached = {}


def _tile_k(t):
    return 128 if t < NT128 else 120


def _build_program():
    from contextlib import ExitStack

    import concourse.bass as bass
    import concourse.mybir as mybir

    bf16 = mybir.dt.bfloat16
    f8e3 = mybir.dt.float8e3
    f32 = mybir.dt.float32

    nc = bass.Bass()
    # w128[p][t*NC + n] = w8T[128t + p, n] for t in [0, 49)
    w128_ext = nc.declare_dram_parameter("w128", [128, NT128 * NC], f8e3,
                                         isOutput=False)
    # w120[p][i*NC + n] = w8T[6272 + 120i + p, n] for i in [0, 16)
    w120_ext = (nc.declare_dram_parameter("w120", [120, NT120 * NC], f8e3,
                                          isOutput=False)
                if NT120 else None)
    # xTp[p, t*M + m] = x[m, krow(t, p)]  (bf16)
    x_ext = nc.declare_dram_parameter("xTp", [128, NTILES * M], bf16,
                                      isOutput=False)
    o_ext = nc.declare_dram_parameter("out", [128, NC], bf16, isOutput=True)

    with ExitStack() as ctx:
        wbuf = ctx.enter_context(nc.sbuf_tensor([128, NTILES * NC], f8e3))
        xbuf = ctx.enter_context(nc.sbuf_tensor([128, NTILES * M], bf16))
        obuf = ctx.enter_context(nc.sbuf_tensor([128, NC], bf16))
        scratch = ctx.enter_context(nc.sbuf_tensor([1, 8], f32))
        ps0 = ctx.enter_context(nc.psum_tensor([128, 512], f32))
        ps1 = ctx.enter_context(nc.psum_tensor([128, 512], f32))
        ps2 = ctx.enter_context(nc.psum_tensor([128, 512], f32))
        xsem = ctx.enter_context(nc.semaphore())
        wsems = [ctx.enter_context(nc.semaphore(name=f"wsem{i}"))
                 for i in range(len(CHUNKS))]
        wtail = ctx.enter_context(nc.semaphore())
        pesem = ctx.enter_context(nc.semaphore())
        vsem = ctx.enter_context(nc.semaphore())
        osem = ctx.enter_context(nc.semaphore())
        block = ctx.enter_context(nc.Block(no_gpsimd_drain=True))

        @block.sync
        def _(sync):
            for c, (kind, t0, nt) in enumerate(CHUNKS):
                if kind == 128:
                    src = w128_ext[:, t0 * NC:(t0 + nt) * NC]
                    dst = wbuf[:, t0 * NC:(t0 + nt) * NC]
                else:
                    i0 = t0 - NT128
                    src = w120_ext[:, i0 * NC:(i0 + nt) * NC]
                    dst = wbuf[0:120, t0 * NC:(t0 + nt) * NC]
                if c == len(CHUNKS) - 1:
                    # finer tail: last chunk lands as two 2-tile pieces
                    half = (nt // 2) * NC
                    sync.dma_start(out=dst[:, 0:half],
                                   in_=src[:, 0:half]).then_inc(wsems[c], 16)
                    sync.dma_start(out=dst[:, half:],
                                   in_=src[:, half:]).then_inc(wtail, 16)
                else:
                    sync.dma_start(out=dst, in_=src).then_inc(wsems[c], 16)
            # bank0 out-DMA as soon as DVE evicted it; bank1 goes out on the
            # scalar queue; wait for both write receipts before retiring
            sync.wait_ge(vsem, 1)
            sync.dma_start(out=o_ext[:, 0:512],
                           in_=obuf[:, 0:512]).then_inc(osem, 16)
            sync.wait_ge(osem, 32)

        @block.scalar
        def _(scalar):
            # x on the ACT HWDGE queue: its completion receipt overlaps
            # chunk0's data on the sync queue
            scalar.dma_start(out=xbuf[:], in_=x_ext[:]).then_inc(xsem, 16)
            # dummy op: pay the ACT table load during the DMA stream
            scalar.copy(scratch[:], scratch[:])
            scalar.wait_ge(pesem, 2)
            scalar.copy(obuf[:, 512:1024], ps1[:])
            scalar.dma_start(out=o_ext[:, 512:1024],
                             in_=obuf[:, 512:1024]).then_inc(osem, 16)

        @block.tensor
        def _(tensor):
            psd = ps2[0:32, :]

            def dummy_mms(n):
                # HAM warm-keepers: fill PE wait-gaps with throwaway matmuls
                # so the activity monitor holds the 2.4 GHz clock.
                for _ in range(n):
                    tensor.matmul(psd, xbuf[:, 0:M], wbuf[:, 0:512],
                                  start=True, stop=True,
                                  tile_position=(0, 0))

            tensor.wait_ge(xsem, 16)
            dummy_mms(8)
            nch = len(CHUNKS)
            for c, (kind, t0, nt) in enumerate(CHUNKS):
                tensor.wait_ge(wsems[c], 16)
                for ti in range(nt):
                    if c == nch - 1 and ti == nt // 2:
                        tensor.wait_ge(wtail, 16)
                    t = t0 + ti
                    kk = _tile_k(t)
                    j = t % 4
                    lhsT = xbuf[0:kk, t * M:(t + 1) * M]
                    w_off = t * NC
                    start = t < 4
                    stop = t >= NTILES - 4
                    mm0 = tensor.matmul(ps0[32 * j:32 * j + 32, :], lhsT,
                                        wbuf[0:kk, w_off:w_off + 512],
                                        start=start, stop=stop,
                                        tile_position=(0, 32 * j))
                    mm1 = tensor.matmul(ps1[32 * j:32 * j + 32, :], lhsT,
                                        wbuf[0:kk, w_off + 512:w_off + 1024],
                                        start=start, stop=stop,
                                        tile_position=(0, 32 * j))
                    if t == NTILES - 1:
                        mm0.then_inc(pesem, 1)
                        mm1.then_inc(pesem, 1)
                if c < nch - 2:
                    dummy_mms(2)

        @block.vector
        def _(vector):
            vector.wait_ge(pesem, 1)
            vector.tensor_copy(obuf[:, 0:512], ps0[:]).then_inc(vsem, 1)

    return nc


def _host_prep(x, packed_weight, scales, zeros):
    """Dequantize, fold per-channel scale, quantize to e3m4, pack layouts."""
    bf16 = ml_dtypes.bfloat16
    e3m4 = ml_dtypes.float8_e3m4
    k = np.arange(K)
    shift = ((k % 2) * 4).astype(np.int32)
    q = ((packed_weight[:, k // 2] >> shift[None, :]) & 15).astype(np.float32)
    g = k // GROUP_SIZE
    w = (q - zeros[:, g]) * scales[:, g]            # [N, K] f32
    S = np.abs(w).max(axis=1) / E3M4_MAX            # [N]
    w8 = (w / S[:, None]).astype(e3m4)              # [N, K] e3m4

    # x^T packed by tile: [128, NTILES*M]
    xTp = np.zeros((128, NTILES * M), np.float32)
    xT = x.T                                        # [K, M]
    for t in range(NTILES):
        kk = _tile_k(t)
        r0 = 128 * t if t < NT128 else ROWS128 + 120 * (t - NT128)
        xTp[0:kk, t * M:(t + 1) * M] = xT[r0:r0 + kk]
    xTp = xTp.astype(bf16)

    in_maps = []
    for c in range(N_CORES):
        wc = w8[c * NC:(c + 1) * NC].T              # [K, NC] e3m4 view
        w128 = np.ascontiguousarray(
            wc[0:ROWS128].reshape(NT128, 128, NC).transpose(1, 0, 2)
              .reshape(128, NT128 * NC))
        m = {"w128": w128, "xTp": xTp}
        if NT120:
            m["w120"] = np.ascontiguousarray(
                wc[ROWS128:].reshape(NT120, 120, NC).transpose(1, 0, 2)
                  .reshape(120, NT120 * NC))
        in_maps.append(m)
    return in_maps, S


def kernel(x, packed_weight, scales, zeros, bias_param, _trace=False):
    from concourse.bass_utils import run_bass_kernel_spmd

    if "nc" not in _cached:
        _cached["nc"] = _build_program()
    nc = _cached["nc"]

    in_maps, S = _host_prep(x, packed_weight, scales, zeros)
    res = run_bass_kernel_spmd(nc, in_maps, core_ids=list(range(N_CORES)),
                               trace=_trace)
    parts = []
    for c in range(N_CORES):
        P = res.results[c]["out"].astype(np.float32)    # [128, NC]
        acc = P[0:32] + P[32:64] + P[64:96] + P[96:128]
        parts.append(acc * S[None, c * NC:(c + 1) * NC])
    out = np.concatenate(parts, axis=1) + bias_param[None, :].astype(np.float32)
    out = out.astype(np.float32, copy=False)
    if _trace:
        return out, res
    return out
